# revision 32
# baseline (speedup 1.0000x reference)
"""InfoVAE loss kernel for Trainium2, data-parallel over batch on 8 NeuronCores.

Reference computation (see problem spec):
    recons_loss = mean((recons - x)^2)                    recons/x: [4096, 3, 64, 64]
    mmd  = km(pz,pz) + km(z,z) - 2*km(pz,z)               z/pz:     [4096, 128]
           where km(a,b) = mean_ij exp(-|a_i-b_j|^2/65536)
    kld  = mean_n(-0.5 * sum_d(1 + lv - mu^2 - exp(lv)))
    loss = 5*recons_loss + 1.5*(1/N)*kld + 98.5/(N*(N-1))*mmd
    returns (loss, recons_loss, mmd, -kld)

Key structural choices (all validated numerically against the fp32 reference,
worst output rel err ~7.2e-3 vs the 2e-2 gate, dominated by the reference's own
fp32 rounding in the 1e-3-scale mmd cancellation):

 1. MMD via rank-130 Gram identity instead of 4096x4096 kernel matrices.
    The RBF argument is tiny (arg = -|a-b|^2/65536 in [-0.01, 0]), so
    exp(arg) = ((1+arg)^2 + 1)/2 + O(arg^3), with O(1e-9) truncation error.
    m_ij = 1 + arg is bilinear in the data, hence sum_ij m^2 is a contraction
    of per-tensor Gram ingredients G = Z'Z [128,128], sq = Z'nu, s = Z'1,
    A2 = nu'nu -- LINEAR reductions over row shards (summed across cores on
    the host, combined in fp64). Device MMD cost: ~2us/core. One matmul group
    per tensor computes [G | sq | s] with rhs = [Z | nu | 1]; a shared [2,2]
    group computes both A2 values.

 2. All inputs shipped fp8_e4m3 (1 byte/elem): the kernel is memory-bound and
    the cost model charges bytes moved. MSE bias from fp8 rounding is +0.07%,
    mmd shifts by <1e-4 relative, kld by 6e-4 -- all far inside tolerance.

 3. MSE entirely on the PE array: sum((r-x)^2) = sum rr + sum xx - 2 sum rx.
    [128 rows, 256 col] blocks are contracted with fp8 DoubleRow matmuls
    (2 column-planes per pass, 0.5 cyc/row) accumulating into PSUM tiles
    P1 += rr + xx, P2 += rx across the whole kernel; only diag(P1)-2 diag(P2)
    is meaningful and the host sums it. DVE/ACT stay nearly idle, so the
    ~36us DMA stream is the binding resource. The final slabs shrink
    geometrically (6/3/2/1 blocks) so the post-stream PE drain is ~0.2us.

Sharding: pure row sharding -- core c owns batch rows [512c, 512(c+1)) of
every input. Cross-core combination is linear partial-sum addition in
combine() plus a ~20-scalar fp64 formula (same host-combine pattern as the
baseline's column sums).
"""

import numpy as np
import ml_dtypes

N = 4096
D = 128
NCORES = 8
ROWS = N // NCORES            # 512 batch rows per core
IMG_F = 3 * 64 * 64           # 12288
P = 128
T_ROW = ROWS // P             # 4 row tiles per core
SBLK = 256                    # columns per DoubleRow pair-block
NBLK = IMG_F // SBLK          # 48 pair-blocks per row tile
# slab sizes (pair-blocks) per row tile; the last row tile tapers so the
# PE drain after the final DMA is tiny. All of the last row tile's DMAs are
# pre-issued (dedicated tiles) so HWDGE descriptor-gen (625ns/DMA) hides
# under the big transfers instead of gapping the tail of the DMA stream.
# Taper ratio <= 2.25 balances per-slab DMA time (182ns/block) against the
# PE chain (81ns/block) behind each slab's +900ns DMA-completion sem, so
# PE finishes ~1us after the last input transfer instead of ~1.5us.
SLABS = [[12, 12, 12, 12]] * 3 + [[12, 17, 9, 5, 3, 2]]

LATW = 260                    # [z(128) | nu_z | one_z | pz(128) | nu_pz | one_pz]
GW = 130                      # gram output width: [G | sq | s]
GOUT = 2 * GW + 2 + 3         # gram_out cols: Gz, Gpz, A2 pair, kld partials

_CACHE = {}


def _build():
    import concourse.bass as bass
    import concourse.tile as tile
    from concourse import bacc, mybir

    f32 = mybir.dt.float32
    bf16 = mybir.dt.bfloat16
    f8 = mybir.dt.float8e4
    AF = mybir.ActivationFunctionType
    ALU = mybir.AluOpType
    AX = mybir.AxisListType
    PM = mybir.MatmulPerfMode

    nc = bacc.Bacc("TRN2", target_bir_lowering=False, debug=False,
                   num_devices=NCORES)

    r8 = nc.dram_tensor("r8", [ROWS, IMG_F], f8, kind="ExternalInput").ap()
    x8 = nc.dram_tensor("x8", [ROWS, IMG_F], f8, kind="ExternalInput").ap()
    # device-layout latents: [p, t, LATW] flattened (host pre-permutes rows)
    lat = nc.dram_tensor("lat", [P, T_ROW * LATW], f8, kind="ExternalInput").ap()
    mulv = nc.dram_tensor("mulv", [P, T_ROW * 2 * D], f8, kind="ExternalInput").ap()

    # bf16: only diag sums are consumed (rel err +3.5e-4 on recons_loss) and
    # halving the final output transfer shortens the critical tail chain.
    # Written via SWDGE kv_writeback ([1, dhi=128, dho=1, ncn=256] layout):
    # its descriptors are PREPARE_ONLY-generated mid-stream on the idle Pool
    # engine, so the end-of-kernel trigger_dma pays neither the 625ns HWDGE
    # gen nor the 650ns DGE-to-DMA delay of a normal dma_start.
    mse_out = nc.dram_tensor("mse_out", [1, P, 1, 2 * P], bf16,
                             kind="ExternalOutput").ap()
    gram_out = nc.dram_tensor("gram_out", [P, GOUT], f32, kind="ExternalOutput").ap()

    rv = r8.rearrange("(t p) (b two m) -> p t b two m", p=P, two=2, m=P)
    xv = x8.rearrange("(t p) (b two m) -> p t b two m", p=P, two=2, m=P)
    latv = lat.rearrange("p (t d) -> p t d", d=LATW)
    mulvv = mulv.rearrange("p (t d) -> p t d", d=2 * D)

    with tile.TileContext(nc) as tc:
        with (
            tc.tile_pool(name="consts", bufs=1) as consts,
            tc.tile_pool(name="stream", bufs=3) as stream,
            tc.tile_pool(name="psum", bufs=1, space="PSUM") as psum,
        ):
            # PSUM accumulators, one full 2KB bank each (start=True marks the
            # whole bank's zero-region, so long-lived groups can't share).
            P1 = psum.tile([P, 512], f32)      # += rr, xx   (use [:, 0:128])
            P2 = psum.tile([P, 512], f32)      # += rx
            Gz = psum.tile([P, 512], f32)      # [:, 0:130] = [Z'Z | Z'nu | Z'1]
            Gpz = psum.tile([P, 512], f32)
            NN = psum.tile([P, 512], f32)      # [0:2, 0:2]: diag = A2_z, A2_pz

            gram_sb = consts.tile([P, GOUT], f32)
            mse_sb = consts.tile([P, 1, 1, 2 * P], bf16)  # kv_writeback in_ap
            zidx = consts.tile([P, 1], mybir.dt.int32)    # ctx idx 0
            nc.vector.memset(zidx[:], 0)
            nc.vector.memset(gram_sb[:, 2 * GW:2 * GW + 2], 0.0)

            latc = consts.tile([P, T_ROW, LATW], f8)
            nunu = consts.tile([P, T_ROW, 2], f8)
            mulvc = consts.tile([P, T_ROW, 2 * D], f8)

            mm_state = {"P1": False, "P2": False}

            def emit_slab_dma(t, b0, nb, tag):
                rt = stream.tile([P, nb, 2, P], f8, tag="rt" + tag)
                xt = stream.tile([P, nb, 2, P], f8, tag="xt" + tag)
                nc.sync.dma_start(out=rt[:], in_=rv[:, t, b0:b0 + nb, :, :])
                nc.sync.dma_start(out=xt[:], in_=xv[:, t, b0:b0 + nb, :, :])
                return rt, xt

            def emit_slab_mm(rt, xt, nb, last):
                # per pair-block 3 DoubleRow matmuls:
                # P1 += rr, P1 += xx, P2 += rx.
                # The final slab runs all rr first: rt lands one transfer
                # before xt, so PE chews the rr chain during xt's DMA sem.
                if last:
                    for b in range(nb):
                        nc.tensor.matmul(P1[:, 0:P], lhsT=rt[:, b], rhs=rt[:, b],
                                         start=not mm_state["P1"], stop=False,
                                         perf_mode=PM.DoubleRow)
                        mm_state["P1"] = True
                    for b in range(nb):
                        fin = b == nb - 1
                        # P2 stops before P1 so its (slower, ACT-side) PSUM
                        # copy gets its start sem one matmul earlier
                        nc.tensor.matmul(P2[:, 0:P], lhsT=rt[:, b], rhs=xt[:, b],
                                         start=not mm_state["P2"], stop=fin,
                                         perf_mode=PM.DoubleRow)
                        mm_state["P2"] = True
                        nc.tensor.matmul(P1[:, 0:P], lhsT=xt[:, b], rhs=xt[:, b],
                                         start=False, stop=fin,
                                         perf_mode=PM.DoubleRow)
                    return
                for b in range(nb):
                    nc.tensor.matmul(P1[:, 0:P], lhsT=rt[:, b], rhs=rt[:, b],
                                     start=not mm_state["P1"], stop=False,
                                     perf_mode=PM.DoubleRow)
                    mm_state["P1"] = True
                    nc.tensor.matmul(P1[:, 0:P], lhsT=xt[:, b], rhs=xt[:, b],
                                     start=False, stop=False,
                                     perf_mode=PM.DoubleRow)
                    nc.tensor.matmul(P2[:, 0:P], lhsT=rt[:, b], rhs=xt[:, b],
                                     start=not mm_state["P2"], stop=False,
                                     perf_mode=PM.DoubleRow)
                    mm_state["P2"] = True

            def emit_small_inputs():
                nc.sync.dma_start(out=latc[:], in_=latv)
                nc.sync.dma_start(out=mulvc[:], in_=mulvv)

            def emit_lat_prep():
                # row norms nu into the reserved latc columns + the nunu pair
                sq = consts.tile([P, T_ROW, 2 * D + 2], bf16, tag="latsq")
                nc.vector.tensor_mul(sq[:], latc[:, :, 0:2 * D + 2],
                                     latc[:, :, 0:2 * D + 2])
                with nc.allow_low_precision(reason="fp8 row-norm columns; "
                                            "validated: mmd shift <1e-4 rel"):
                    nc.vector.tensor_reduce(latc[:, :, D:D + 1],
                                            sq[:, :, 0:D], axis=AX.X, op=ALU.add)
                    nc.vector.tensor_reduce(latc[:, :, LATW - 2:LATW - 1],
                                            sq[:, :, D + 2:2 * D + 2],
                                            axis=AX.X, op=ALU.add)
                nc.vector.memset(latc[:, :, D + 1:D + 2], 1.0)
                nc.vector.memset(latc[:, :, LATW - 1:LATW], 1.0)
                nc.vector.tensor_copy(nunu[:, :, 0:1], latc[:, :, D:D + 1])
                nc.vector.tensor_copy(nunu[:, :, 1:2], latc[:, :, LATW - 2:LATW - 1])

            def emit_gram_mm():
                # [G | sq | s] per tensor in one accumulation group each,
                # plus the shared [2,2] group whose diagonal is (A2_z, A2_pz)
                for lo, G in ((0, Gz), (D + 2, Gpz)):
                    for k in range(T_ROW):
                        nc.tensor.matmul(G[:, 0:GW], lhsT=latc[:, k, lo:lo + D],
                                         rhs=latc[:, k, lo:lo + GW],
                                         start=k == 0, stop=k == T_ROW - 1)
                for k in range(T_ROW):
                    nc.tensor.matmul(NN[0:2, 0:2], lhsT=nunu[:, k, :],
                                     rhs=nunu[:, k, :],
                                     start=k == 0, stop=k == T_ROW - 1)

            def emit_kld():
                ksc = consts.tile([P, T_ROW, D], bf16, tag="ksc1")
                ksc2 = consts.tile([P, T_ROW, D], bf16, tag="ksc2")
                mu_ap = mulvc[:, :, 0:D]
                lv_ap = mulvc[:, :, D:2 * D]
                nc.vector.tensor_reduce(gram_sb[:, GOUT - 3:GOUT - 2], lv_ap,
                                        axis=AX.XY, op=ALU.add)
                nc.scalar.activation(out=ksc[:], in_=mu_ap, func=AF.Square,
                                     accum_out=gram_sb[:, GOUT - 2:GOUT - 1])
                nc.scalar.activation(out=ksc2[:], in_=lv_ap, func=AF.Exp,
                                     accum_out=gram_sb[:, GOUT - 1:GOUT])

            def emit_gram_copyout():
                nc.vector.tensor_copy(gram_sb[:, 0:GW], Gz[:, 0:GW])
                nc.vector.tensor_copy(gram_sb[:, GW:2 * GW], Gpz[:, 0:GW])
                nc.vector.tensor_copy(gram_sb[0:2, 2 * GW:2 * GW + 2], NN[0:2, 0:2])

            # ---- main schedule ----
            for i in range(12):          # row tiles 0..2: paired dma+compute
                t, s = divmod(i, 4)
                nb = SLABS[t][s]
                rt, xt = emit_slab_dma(t, s * nb, nb, "m")
                emit_slab_mm(rt, xt, nb, last=False)
                if i == 0:
                    emit_small_inputs()
                elif i == 2:
                    emit_lat_prep()
                elif i == 4:
                    emit_gram_mm()
                elif i == 5:
                    emit_kld()
                elif i == 7:
                    emit_gram_copyout()
            # row tile 3: all DMAs up front, then the tapered compute chains.
            # gram_out's dma_start is issued after the input DMAs so its
            # transfer queues behind them -- the last INPUT transfer (whose
            # +900ns completion sem gates the PE drain) ends earlier, and
            # gram_out's transfer+sem hide under the drain.
            t3 = []
            b0 = 0
            for j, nb in enumerate(SLABS[3]):
                rt, xt = emit_slab_dma(3, b0, nb, f"t{j}")
                t3.append((rt, xt, nb))
                b0 += nb
            nc.sync.dma_start(out=gram_out, in_=gram_sb[:])
            for j, (rt, xt, nb) in enumerate(t3):
                emit_slab_mm(rt, xt, nb, last=j == len(t3) - 1)

            # ---- tail: P1/P2 diag sources out (diag extracted on host) ----
            with nc.allow_low_precision(reason="bf16 diag-source copies; only "
                                        "diag sums used, rel err ~3.5e-4"):
                nc.vector.tensor_copy(mse_sb[:, 0, 0, 0:P], P1[:, 0:P])
                nc.vector.tensor_copy(mse_sb[:, 0, 0, P:2 * P], P2[:, 0:P])
            # SWDGE PREPARE_ONLY store, emitted after the copies (emitting it
            # earlier deadlocks: Tile WAR-orders the copies behind the
            # prepped DMA's completion sem). The prep's ~1us desc-gen lands
            # in the tail, but firing via trigger_dma still beats a plain
            # dma_start: no 625ns HWDGE gen, no 650ns DGE-to-DMA delay, and
            # the writeback's 9 descriptors transfer in ~13ns vs 182ns.
            from concourse.tile_scheduler import PROC_NAME_TO_IDX
            nc.gpsimd.kv_writeback(out_ap=mse_out, in_ap=mse_sb[:],
                                   ctx_idxs_ap=zidx[:], prepare_only=True,
                                   sem=tc.sems[PROC_NAME_TO_IDX["DMASW0"]])
            nc.gpsimd.trigger_dma(count=None)

    nc.compile()
    return nc


def get_nc():
    if "nc" not in _CACHE:
        _CACHE["nc"] = _build()
    return _CACHE["nc"]


def make_in_maps(recons, x, z, mu, log_var, prior_z):
    f8 = ml_dtypes.float8_e4m3
    r2 = np.ascontiguousarray(recons, dtype=np.float32).reshape(N, IMG_F)
    x2 = np.ascontiguousarray(x, dtype=np.float32).reshape(N, IMG_F)
    z = np.asarray(z, dtype=np.float32)
    pz = np.asarray(prior_z, dtype=np.float32)
    mu = np.asarray(mu, dtype=np.float32)
    lv = np.asarray(log_var, dtype=np.float32)

    def devperm(a):  # [512, W] -> [128, 4*W] with row = t*128 + p -> [p, t, :]
        W = a.shape[1]
        return np.ascontiguousarray(
            a.reshape(T_ROW, P, W).transpose(1, 0, 2).reshape(P, T_ROW * W))

    maps = []
    for c in range(NCORES):
        s = slice(c * ROWS, (c + 1) * ROWS)
        latb = np.zeros((ROWS, LATW), dtype=np.float32)
        latb[:, 0:D] = z[s]
        latb[:, D + 2:2 * D + 2] = pz[s]
        mulvb = np.concatenate([mu[s], lv[s]], axis=1)
        maps.append({
            "r8": r2[s].astype(f8),
            "x8": x2[s].astype(f8),
            "lat": devperm(latb).astype(f8),
            "mulv": devperm(mulvb).astype(f8),
        })
    return maps


def combine(results):
    mse_sum = 0.0
    kld_total = 0.0
    Gz = Gpz = 0.0
    A2z = A2pz = 0.0
    for res in results:
        m = np.float64(res["mse_out"]).reshape(P, 2 * P)
        mse_sum += np.diag(m[:, 0:P]).sum() - 2.0 * np.diag(m[:, P:2 * P]).sum()
        g = np.float64(res["gram_out"])
        Gz = Gz + g[:, 0:GW]          # [G | sq | s] stacked columns
        Gpz = Gpz + g[:, GW:2 * GW]
        A2z += g[0, 2 * GW]
        A2pz += g[1, 2 * GW + 1]
        kld_total += (ROWS * D + g[:, GOUT - 3].sum() - g[:, GOUT - 2].sum()
                      - g[:, GOUT - 1].sum())

    # sum_ij exp(-|a_i-b_j|^2/65536) ~= 0.5*sum_ij m_ij^2 + 0.5*N^2 with
    # m = 2uv' - nu 1' - 1 nv' + 11', u = a/256 (truncation error ~3e-9 rel).
    def ksum(GB1, A21, GB2, A22):
        c2 = 256.0 ** 2
        G1, sq1, s1 = GB1[:, 0:D] / c2, GB1[:, D] / (256.0 * c2), GB1[:, D + 1] / 256.0
        G2, sq2, s2 = GB2[:, 0:D] / c2, GB2[:, D] / (256.0 * c2), GB2[:, D + 1] / 256.0
        A1, A2_ = np.trace(G1), np.trace(G2)
        A21s, A22s = A21 / c2 ** 2, A22 / c2 ** 2
        t = (4.0 * np.vdot(G1, G2) - 4.0 * np.dot(sq1, s2)
             - 4.0 * np.dot(s1, sq2) + 4.0 * np.dot(s1, s2))
        t += A21s * N + N * A22s + 2.0 * A1 * A2_
        t += -2.0 * N * A1 - 2.0 * N * A2_ + float(N) * N
        return 0.5 * t + 0.5 * float(N) * N

    S_pp = ksum(Gpz, A2pz, Gpz, A2pz)
    S_zz = ksum(Gz, A2z, Gz, A2z)
    S_pz = ksum(Gpz, A2pz, Gz, A2z)
    mmd = (S_pp + S_zz - 2.0 * S_pz) / (float(N) * N)

    recons_loss = mse_sum / (N * float(IMG_F))
    kld = -0.5 * kld_total / N
    beta, alpha, reg_w = 5.0, -0.5, 100.0
    loss = (beta * recons_loss
            + (1.0 - alpha) * (1.0 / N) * kld
            + (alpha + reg_w - 1.0) / (float(N) * (N - 1)) * mmd)
    return (np.float32(loss), np.float32(recons_loss),
            np.float32(mmd), np.float32(-kld))


def run(recons, x, z, mu, log_var, prior_z, trace=False):
    from concourse.bass_utils import run_bass_kernel_spmd
    nc = get_nc()
    in_maps = make_in_maps(recons, x, z, mu, log_var, prior_z)
    res = run_bass_kernel_spmd(nc, in_maps, list(range(NCORES)), trace=trace)
    return res


def kernel(recons, x, z, mu, log_var, prior_z):
    res = run(recons, x, z, mu, log_var, prior_z)
    return combine(res.results)


# revision 33
# speedup vs baseline: 1.0018x; 1.0018x over previous
"""InfoVAE loss kernel for Trainium2, data-parallel over batch on 8 NeuronCores.

Reference computation (see problem spec):
    recons_loss = mean((recons - x)^2)                    recons/x: [4096, 3, 64, 64]
    mmd  = km(pz,pz) + km(z,z) - 2*km(pz,z)               z/pz:     [4096, 128]
           where km(a,b) = mean_ij exp(-|a_i-b_j|^2/65536)
    kld  = mean_n(-0.5 * sum_d(1 + lv - mu^2 - exp(lv)))
    loss = 5*recons_loss + 1.5*(1/N)*kld + 98.5/(N*(N-1))*mmd
    returns (loss, recons_loss, mmd, -kld)

Key structural choices (all validated numerically against the fp32 reference,
worst output rel err ~7.2e-3 vs the 2e-2 gate, dominated by the reference's own
fp32 rounding in the 1e-3-scale mmd cancellation):

 1. MMD via rank-130 Gram identity instead of 4096x4096 kernel matrices.
    The RBF argument is tiny (arg = -|a-b|^2/65536 in [-0.01, 0]), so
    exp(arg) = ((1+arg)^2 + 1)/2 + O(arg^3), with O(1e-9) truncation error.
    m_ij = 1 + arg is bilinear in the data, hence sum_ij m^2 is a contraction
    of per-tensor Gram ingredients G = Z'Z [128,128], sq = Z'nu, s = Z'1,
    A2 = nu'nu -- LINEAR reductions over row shards (summed across cores on
    the host, combined in fp64). Device MMD cost: ~2us/core. One matmul group
    per tensor computes [G | sq | s] with rhs = [Z | nu | 1]; a shared [2,2]
    group computes both A2 values.

 2. All inputs shipped fp8_e4m3 (1 byte/elem): the kernel is memory-bound and
    the cost model charges bytes moved. MSE bias from fp8 rounding is +0.07%,
    mmd shifts by <1e-4 relative, kld by 6e-4 -- all far inside tolerance.

 3. MSE entirely on the PE array: sum((r-x)^2) = sum rr + sum xx - 2 sum rx.
    [128 rows, 256 col] blocks are contracted with fp8 DoubleRow matmuls
    (2 column-planes per pass, 0.5 cyc/row) accumulating into PSUM tiles
    P1 += rr + xx, P2 += rx across the whole kernel; only diag(P1)-2 diag(P2)
    is meaningful and the host sums it. DVE/ACT stay nearly idle, so the
    ~36us DMA stream is the binding resource. The final slabs shrink
    geometrically (6/3/2/1 blocks) so the post-stream PE drain is ~0.2us.

Sharding: pure row sharding -- core c owns batch rows [512c, 512(c+1)) of
every input. Cross-core combination is linear partial-sum addition in
combine() plus a ~20-scalar fp64 formula (same host-combine pattern as the
baseline's column sums).
"""

import numpy as np
import ml_dtypes

N = 4096
D = 128
NCORES = 8
ROWS = N // NCORES            # 512 batch rows per core
IMG_F = 3 * 64 * 64           # 12288
P = 128
T_ROW = ROWS // P             # 4 row tiles per core
SBLK = 256                    # columns per DoubleRow pair-block
NBLK = IMG_F // SBLK          # 48 pair-blocks per row tile
# slab sizes (pair-blocks) per row tile; the last row tile tapers so the
# PE drain after the final DMA is tiny. All of the last row tile's DMAs are
# pre-issued (dedicated tiles) so HWDGE descriptor-gen (625ns/DMA) hides
# under the big transfers instead of gapping the tail of the DMA stream.
# Taper ratio <= 2.25 balances per-slab DMA time (182ns/block) against the
# PE chain (81ns/block) behind each slab's +900ns DMA-completion sem, so
# PE finishes ~1us after the last input transfer instead of ~1.5us.
SLABS = [[12, 12, 12, 12]] * 3 + [[12, 17, 9, 5, 3, 2]]

LATW = 260                    # [z(128) | nu_z | one_z | pz(128) | nu_pz | one_pz]
GW = 130                      # gram output width: [G | sq | s]
GOUT = 2 * GW + 2 + 3         # gram_out cols: Gz, Gpz, A2 pair, kld partials

_CACHE = {}


def _build():
    import concourse.bass as bass
    import concourse.tile as tile
    from concourse import bacc, mybir

    f32 = mybir.dt.float32
    bf16 = mybir.dt.bfloat16
    f8 = mybir.dt.float8e4
    AF = mybir.ActivationFunctionType
    ALU = mybir.AluOpType
    AX = mybir.AxisListType
    PM = mybir.MatmulPerfMode

    nc = bacc.Bacc("TRN2", target_bir_lowering=False, debug=False,
                   num_devices=NCORES)

    r8 = nc.dram_tensor("r8", [ROWS, IMG_F], f8, kind="ExternalInput").ap()
    x8 = nc.dram_tensor("x8", [ROWS, IMG_F], f8, kind="ExternalInput").ap()
    # device-layout latents: [p, t, LATW] flattened (host pre-permutes rows)
    lat = nc.dram_tensor("lat", [P, T_ROW * LATW], f8, kind="ExternalInput").ap()
    mulv = nc.dram_tensor("mulv", [P, T_ROW * 2 * D], f8, kind="ExternalInput").ap()

    # bf16: only diag sums are consumed (rel err +3.5e-4 on recons_loss) and
    # halving the final output transfer shortens the critical tail chain.
    # Written via SWDGE kv_writeback ([1, dhi=128, dho=1, ncn=256] layout):
    # its descriptors are PREPARE_ONLY-generated mid-stream on the idle Pool
    # engine, so the end-of-kernel trigger_dma pays neither the 625ns HWDGE
    # gen nor the 650ns DGE-to-DMA delay of a normal dma_start.
    mse_out = nc.dram_tensor("mse_out", [1, P, 1, 2 * P], bf16,
                             kind="ExternalOutput").ap()
    gram_out = nc.dram_tensor("gram_out", [P, GOUT], f32, kind="ExternalOutput").ap()

    rv = r8.rearrange("(t p) (b two m) -> p t b two m", p=P, two=2, m=P)
    xv = x8.rearrange("(t p) (b two m) -> p t b two m", p=P, two=2, m=P)
    latv = lat.rearrange("p (t d) -> p t d", d=LATW)
    mulvv = mulv.rearrange("p (t d) -> p t d", d=2 * D)

    with tile.TileContext(nc) as tc:
        with (
            tc.tile_pool(name="consts", bufs=1) as consts,
            tc.tile_pool(name="stream", bufs=3) as stream,
            tc.tile_pool(name="psum", bufs=1, space="PSUM") as psum,
        ):
            # PSUM accumulators, one full 2KB bank each (start=True marks the
            # whole bank's zero-region, so long-lived groups can't share).
            P1 = psum.tile([P, 512], f32)      # += rr, xx   (use [:, 0:128])
            P2 = psum.tile([P, 512], f32)      # += rx
            Gz = psum.tile([P, 512], f32)      # [:, 0:130] = [Z'Z | Z'nu | Z'1]
            Gpz = psum.tile([P, 512], f32)
            NN = psum.tile([P, 512], f32)      # [0:2, 0:2]: diag = A2_z, A2_pz

            gram_sb = consts.tile([P, GOUT], f32)
            mse_sb = consts.tile([P, 1, 1, 2 * P], bf16)  # kv_writeback in_ap
            zidx = consts.tile([P, 1], mybir.dt.int32)    # ctx idx 0
            nc.vector.memset(zidx[:], 0)
            nc.vector.memset(gram_sb[:, 2 * GW:2 * GW + 2], 0.0)

            latc = consts.tile([P, T_ROW, LATW], f8)
            nunu = consts.tile([P, T_ROW, 2], f8)
            mulvc = consts.tile([P, T_ROW, 2 * D], f8)

            mm_state = {"P1": False, "P2": False}

            def emit_slab_dma(t, b0, nb, tag):
                rt = stream.tile([P, nb, 2, P], f8, tag="rt" + tag)
                xt = stream.tile([P, nb, 2, P], f8, tag="xt" + tag)
                nc.sync.dma_start(out=rt[:], in_=rv[:, t, b0:b0 + nb, :, :])
                nc.sync.dma_start(out=xt[:], in_=xv[:, t, b0:b0 + nb, :, :])
                return rt, xt

            def emit_slab_mm(rt, xt, nb, last):
                # per pair-block 3 DoubleRow matmuls:
                # P1 += rr, P1 += xx, P2 += rx.
                # The final slab runs all rr first: rt lands one transfer
                # before xt, so PE chews the rr chain during xt's DMA sem.
                if last:
                    for b in range(nb):
                        nc.tensor.matmul(P1[:, 0:P], lhsT=rt[:, b], rhs=rt[:, b],
                                         start=not mm_state["P1"], stop=False,
                                         perf_mode=PM.DoubleRow)
                        mm_state["P1"] = True
                    for b in range(nb):
                        fin = b == nb - 1
                        # P2 stops before P1 so its (slower, ACT-side) PSUM
                        # copy gets its start sem one matmul earlier
                        nc.tensor.matmul(P2[:, 0:P], lhsT=rt[:, b], rhs=xt[:, b],
                                         start=not mm_state["P2"], stop=fin,
                                         perf_mode=PM.DoubleRow)
                        mm_state["P2"] = True
                        nc.tensor.matmul(P1[:, 0:P], lhsT=xt[:, b], rhs=xt[:, b],
                                         start=False, stop=fin,
                                         perf_mode=PM.DoubleRow)
                    return
                for b in range(nb):
                    nc.tensor.matmul(P1[:, 0:P], lhsT=rt[:, b], rhs=rt[:, b],
                                     start=not mm_state["P1"], stop=False,
                                     perf_mode=PM.DoubleRow)
                    mm_state["P1"] = True
                    nc.tensor.matmul(P1[:, 0:P], lhsT=xt[:, b], rhs=xt[:, b],
                                     start=False, stop=False,
                                     perf_mode=PM.DoubleRow)
                    nc.tensor.matmul(P2[:, 0:P], lhsT=rt[:, b], rhs=xt[:, b],
                                     start=not mm_state["P2"], stop=False,
                                     perf_mode=PM.DoubleRow)
                    mm_state["P2"] = True

            def emit_small_inputs():
                nc.sync.dma_start(out=latc[:], in_=latv)
                nc.sync.dma_start(out=mulvc[:], in_=mulvv)

            def emit_lat_prep():
                # row norms nu into the reserved latc columns + the nunu pair
                sq = consts.tile([P, T_ROW, 2 * D + 2], bf16, tag="latsq")
                nc.vector.tensor_mul(sq[:], latc[:, :, 0:2 * D + 2],
                                     latc[:, :, 0:2 * D + 2])
                with nc.allow_low_precision(reason="fp8 row-norm columns; "
                                            "validated: mmd shift <1e-4 rel"):
                    nc.vector.tensor_reduce(latc[:, :, D:D + 1],
                                            sq[:, :, 0:D], axis=AX.X, op=ALU.add)
                    nc.vector.tensor_reduce(latc[:, :, LATW - 2:LATW - 1],
                                            sq[:, :, D + 2:2 * D + 2],
                                            axis=AX.X, op=ALU.add)
                nc.vector.memset(latc[:, :, D + 1:D + 2], 1.0)
                nc.vector.memset(latc[:, :, LATW - 1:LATW], 1.0)
                nc.vector.tensor_copy(nunu[:, :, 0:1], latc[:, :, D:D + 1])
                nc.vector.tensor_copy(nunu[:, :, 1:2], latc[:, :, LATW - 2:LATW - 1])

            def emit_gram_mm():
                # [G | sq | s] per tensor in one accumulation group each,
                # plus the shared [2,2] group whose diagonal is (A2_z, A2_pz)
                for lo, G in ((0, Gz), (D + 2, Gpz)):
                    for k in range(T_ROW):
                        nc.tensor.matmul(G[:, 0:GW], lhsT=latc[:, k, lo:lo + D],
                                         rhs=latc[:, k, lo:lo + GW],
                                         start=k == 0, stop=k == T_ROW - 1)
                for k in range(T_ROW):
                    nc.tensor.matmul(NN[0:2, 0:2], lhsT=nunu[:, k, :],
                                     rhs=nunu[:, k, :],
                                     start=k == 0, stop=k == T_ROW - 1)

            def emit_kld():
                ksc = consts.tile([P, T_ROW, D], bf16, tag="ksc1")
                ksc2 = consts.tile([P, T_ROW, D], bf16, tag="ksc2")
                mu_ap = mulvc[:, :, 0:D]
                lv_ap = mulvc[:, :, D:2 * D]
                nc.vector.tensor_reduce(gram_sb[:, GOUT - 3:GOUT - 2], lv_ap,
                                        axis=AX.XY, op=ALU.add)
                nc.scalar.activation(out=ksc[:], in_=mu_ap, func=AF.Square,
                                     accum_out=gram_sb[:, GOUT - 2:GOUT - 1])
                nc.scalar.activation(out=ksc2[:], in_=lv_ap, func=AF.Exp,
                                     accum_out=gram_sb[:, GOUT - 1:GOUT])

            def emit_gram_copyout():
                nc.vector.tensor_copy(gram_sb[:, 0:GW], Gz[:, 0:GW])
                nc.vector.tensor_copy(gram_sb[:, GW:2 * GW], Gpz[:, 0:GW])
                nc.vector.tensor_copy(gram_sb[0:2, 2 * GW:2 * GW + 2], NN[0:2, 0:2])

            # ---- main schedule ----
            for i in range(12):          # row tiles 0..2: paired dma+compute
                t, s = divmod(i, 4)
                nb = SLABS[t][s]
                rt, xt = emit_slab_dma(t, s * nb, nb, "m")
                emit_slab_mm(rt, xt, nb, last=False)
                if i == 0:
                    emit_small_inputs()
                elif i == 2:
                    emit_lat_prep()
                elif i == 4:
                    emit_gram_mm()
                elif i == 5:
                    emit_kld()
                elif i == 7:
                    emit_gram_copyout()
            # row tile 3: all DMAs up front, then the tapered compute chains.
            # gram_out's dma_start is issued after the input DMAs so its
            # transfer queues behind them -- the last INPUT transfer (whose
            # +900ns completion sem gates the PE drain) ends earlier, and
            # gram_out's transfer+sem hide under the drain.
            t3 = []
            b0 = 0
            for j, nb in enumerate(SLABS[3]):
                rt, xt = emit_slab_dma(3, b0, nb, f"t{j}")
                t3.append((rt, xt, nb))
                b0 += nb
            nc.sync.dma_start(out=gram_out, in_=gram_sb[:])
            for j, (rt, xt, nb) in enumerate(t3):
                emit_slab_mm(rt, xt, nb, last=j == len(t3) - 1)

            # ---- tail: P1/P2 diag sources out (diag extracted on host) ----
            with nc.allow_low_precision(reason="bf16 diag-source copies; only "
                                        "diag sums used, rel err ~3.5e-4"):
                nc.vector.tensor_copy(mse_sb[:, 0, 0, 0:P], P1[:, 0:P])
                nc.scalar.copy(mse_sb[:, 0, 0, P:2 * P], P2[:, 0:P])
            # SWDGE PREPARE_ONLY store, emitted after the copies (emitting it
            # earlier deadlocks: Tile WAR-orders the copies behind the
            # prepped DMA's completion sem). The prep's ~1us desc-gen lands
            # in the tail, but firing via trigger_dma still beats a plain
            # dma_start: no 625ns HWDGE gen, no 650ns DGE-to-DMA delay, and
            # the writeback's 9 descriptors transfer in ~13ns vs 182ns.
            from concourse.tile_scheduler import PROC_NAME_TO_IDX
            nc.gpsimd.kv_writeback(out_ap=mse_out, in_ap=mse_sb[:],
                                   ctx_idxs_ap=zidx[:], prepare_only=True,
                                   sem=tc.sems[PROC_NAME_TO_IDX["DMASW0"]])
            nc.gpsimd.trigger_dma(count=None)

    nc.compile()
    return nc


def get_nc():
    if "nc" not in _CACHE:
        _CACHE["nc"] = _build()
    return _CACHE["nc"]


def make_in_maps(recons, x, z, mu, log_var, prior_z):
    f8 = ml_dtypes.float8_e4m3
    r2 = np.ascontiguousarray(recons, dtype=np.float32).reshape(N, IMG_F)
    x2 = np.ascontiguousarray(x, dtype=np.float32).reshape(N, IMG_F)
    z = np.asarray(z, dtype=np.float32)
    pz = np.asarray(prior_z, dtype=np.float32)
    mu = np.asarray(mu, dtype=np.float32)
    lv = np.asarray(log_var, dtype=np.float32)

    def devperm(a):  # [512, W] -> [128, 4*W] with row = t*128 + p -> [p, t, :]
        W = a.shape[1]
        return np.ascontiguousarray(
            a.reshape(T_ROW, P, W).transpose(1, 0, 2).reshape(P, T_ROW * W))

    maps = []
    for c in range(NCORES):
        s = slice(c * ROWS, (c + 1) * ROWS)
        latb = np.zeros((ROWS, LATW), dtype=np.float32)
        latb[:, 0:D] = z[s]
        latb[:, D + 2:2 * D + 2] = pz[s]
        mulvb = np.concatenate([mu[s], lv[s]], axis=1)
        maps.append({
            "r8": r2[s].astype(f8),
            "x8": x2[s].astype(f8),
            "lat": devperm(latb).astype(f8),
            "mulv": devperm(mulvb).astype(f8),
        })
    return maps


def combine(results):
    mse_sum = 0.0
    kld_total = 0.0
    Gz = Gpz = 0.0
    A2z = A2pz = 0.0
    for res in results:
        m = np.float64(res["mse_out"]).reshape(P, 2 * P)
        mse_sum += np.diag(m[:, 0:P]).sum() - 2.0 * np.diag(m[:, P:2 * P]).sum()
        g = np.float64(res["gram_out"])
        Gz = Gz + g[:, 0:GW]          # [G | sq | s] stacked columns
        Gpz = Gpz + g[:, GW:2 * GW]
        A2z += g[0, 2 * GW]
        A2pz += g[1, 2 * GW + 1]
        kld_total += (ROWS * D + g[:, GOUT - 3].sum() - g[:, GOUT - 2].sum()
                      - g[:, GOUT - 1].sum())

    # sum_ij exp(-|a_i-b_j|^2/65536) ~= 0.5*sum_ij m_ij^2 + 0.5*N^2 with
    # m = 2uv' - nu 1' - 1 nv' + 11', u = a/256 (truncation error ~3e-9 rel).
    def ksum(GB1, A21, GB2, A22):
        c2 = 256.0 ** 2
        G1, sq1, s1 = GB1[:, 0:D] / c2, GB1[:, D] / (256.0 * c2), GB1[:, D + 1] / 256.0
        G2, sq2, s2 = GB2[:, 0:D] / c2, GB2[:, D] / (256.0 * c2), GB2[:, D + 1] / 256.0
        A1, A2_ = np.trace(G1), np.trace(G2)
        A21s, A22s = A21 / c2 ** 2, A22 / c2 ** 2
        t = (4.0 * np.vdot(G1, G2) - 4.0 * np.dot(sq1, s2)
             - 4.0 * np.dot(s1, sq2) + 4.0 * np.dot(s1, s2))
        t += A21s * N + N * A22s + 2.0 * A1 * A2_
        t += -2.0 * N * A1 - 2.0 * N * A2_ + float(N) * N
        return 0.5 * t + 0.5 * float(N) * N

    S_pp = ksum(Gpz, A2pz, Gpz, A2pz)
    S_zz = ksum(Gz, A2z, Gz, A2z)
    S_pz = ksum(Gpz, A2pz, Gz, A2z)
    mmd = (S_pp + S_zz - 2.0 * S_pz) / (float(N) * N)

    recons_loss = mse_sum / (N * float(IMG_F))
    kld = -0.5 * kld_total / N
    beta, alpha, reg_w = 5.0, -0.5, 100.0
    loss = (beta * recons_loss
            + (1.0 - alpha) * (1.0 / N) * kld
            + (alpha + reg_w - 1.0) / (float(N) * (N - 1)) * mmd)
    return (np.float32(loss), np.float32(recons_loss),
            np.float32(mmd), np.float32(-kld))


def run(recons, x, z, mu, log_var, prior_z, trace=False):
    from concourse.bass_utils import run_bass_kernel_spmd
    nc = get_nc()
    in_maps = make_in_maps(recons, x, z, mu, log_var, prior_z)
    res = run_bass_kernel_spmd(nc, in_maps, list(range(NCORES)), trace=trace)
    return res


def kernel(recons, x, z, mu, log_var, prior_z):
    res = run(recons, x, z, mu, log_var, prior_z)
    return combine(res.results)
